# revision 1
# baseline (speedup 1.0000x reference)
"""Trainium2 Bass kernel for nn_MultiHeadAttention (B=8192, D=1024, 16 heads
used only via the softmax scale 1/8).

Strategy (8 NeuronCores, zero inter-core communication):
  - Rows (batch axis) of the attention output are sharded: core c owns rows
    [c*1024, (c+1)*1024).
  - Every core recomputes the full K^T and V projections for all 8192 rows
    (replicated compute instead of an all-gather; collectives on this part
    are slower than the 2x17 GFLOP of extra matmul).
  - Attention runs in a transposed-energy ("E^T") layout so no probability
    transpose is ever needed:
        E^T[j, i] = sum_o K^T[o, j] * Q^T[o, i]
        P^T = exp(E^T * 0.125)            (no max subtraction; |E|<40, safe)
        out_unnorm[i, o] = sum_j P^T[j, i] * V[j, o]
        s[i] = sum_j P^T[j, i]            (matmul against a ones vector)
        out = out_unnorm / s + bv         (bv folded in post-normalization)
  - All big matmuls run in float32r (full-rate streaming on the PE at
    N=512) with fp32 PSUM accumulation.
"""

import sys

sys.path.insert(0, "/opt/trn_rl_repo")

import numpy as np

import concourse.bass as bass  # noqa: F401
import concourse.tile as tile
from concourse import bacc, mybir
from concourse.bass_utils import run_bass_kernel_spmd
from concourse.masks import make_identity

B = 8192
D = 1024
P = 128
NCORES = 8
R = B // NCORES  # 1024 rows per core
JBLK = 512  # j-block (keys/values) streamed per iteration
NJB = B // JBLK  # 16
DO = D // P  # 8 feature chunks of 128
IC = R // P  # 8 row chunks of 128 per core
F32 = mybir.dt.float32
F32R = mybir.dt.float32r
BF16 = mybir.dt.bfloat16
AF = mybir.ActivationFunctionType
ALU = mybir.AluOpType
SCALE = 0.125  # 1/sqrt(head_dim=64)




def _transpose_rows_to_sbuf(nc, tp_psum, identity, row_sb, xt_dst, jj):
    """PE-transpose a [128, 1024] natural-layout row chunk into the
    [128(d_in), DO, ...] transposed SBUF tile at free offset jj*128."""
    for dd in range(DO):
        tp = tp_psum.tile([P, P], F32, tag="tp")
        nc.tensor.transpose(tp, row_sb[:, dd * P : (dd + 1) * P], identity)
        nc.vector.tensor_copy(
            out=xt_dst[:, dd, jj * P : (jj + 1) * P], in_=tp
        )


def build_program():
    nc = bacc.Bacc(
        "TRN2", target_bir_lowering=False, debug=False, num_devices=NCORES
    )
    x = nc.dram_tensor("x", [B, D], F32, kind="ExternalInput").ap()
    x_loc = nc.dram_tensor("x_loc", [R, D], F32, kind="ExternalInput").ap()
    w_q = nc.dram_tensor("Wq", [D, D], F32, kind="ExternalInput").ap()
    w_k = nc.dram_tensor("Wk", [D, D], F32, kind="ExternalInput").ap()
    w_v = nc.dram_tensor("Wv", [D, D], F32, kind="ExternalInput").ap()
    b_q = nc.dram_tensor("bq", [D], F32, kind="ExternalInput").ap()
    b_k = nc.dram_tensor("bk", [D], F32, kind="ExternalInput").ap()
    b_v = nc.dram_tensor("bv", [D], F32, kind="ExternalInput").ap()
    out_loc = nc.dram_tensor("out_loc", [R, D], F32, kind="ExternalOutput").ap()

    with tile.TileContext(nc) as tc:
        _body(nc, tc, x, x_loc, w_q, w_k, w_v, b_q, b_k, b_v, out_loc)
    nc.compile()
    return nc


def _body(nc, tc, x, x_loc, w_q, w_k, w_v, b_q, b_k, b_v, out_loc):
    from contextlib import ExitStack

    outer = ExitStack()
    outer.__enter__()
    # ---- persistent pools (whole kernel) ----
    const_pool = outer.enter_context(tc.tile_pool(name="const", bufs=1))
    identity = const_pool.tile([P, P], F32)
    make_identity(nc, identity)
    ones_f32 = const_pool.tile([P, 2], F32)
    nc.vector.memset(ones_f32, 1.0)
    ones = const_pool.tile([P, 2], BF16)
    nc.vector.tensor_copy(out=ones, in_=ones_f32)
    bq_sb = const_pool.tile([P, DO], F32)
    nc.sync.dma_start(bq_sb, b_q.rearrange("(oo p) -> p oo", p=P))
    bk_sb = const_pool.tile([P, DO], F32)
    nc.sync.dma_start(bk_sb, b_k.rearrange("(oo p) -> p oo", p=P))
    ones_row = const_pool.tile([1, P], F32)
    nc.vector.memset(ones_row, 1.0)
    # broadcast bv across all 128 partitions with a K=1 matmul:
    # load bv into partition 0 of bv_bc, then out[p, o] = 1 * bv[o]
    bv_bc = const_pool.tile([P, D], F32)
    nc.sync.dma_start(bv_bc[0:1, :], b_v[None, :])
    with tc.tile_pool(name="bv_psum", bufs=2, space="PSUM") as bvp:
        for oh in range(2):
            pt = bvp.tile([P, 512], F32, tag="bvp")
            nc.tensor.matmul(
                pt,
                ones_row,
                bv_bc[0:1, oh * 512 : (oh + 1) * 512],
                start=True,
                stop=True,
            )
            nc.vector.tensor_copy(out=bv_bc[:, oh * 512 : (oh + 1) * 512], in_=pt)

    qt_pool = outer.enter_context(tc.tile_pool(name="qt", bufs=1))
    qt = qt_pool.tile([P, DO, R], F32R)  # Q^T: [o_in, o_out, i]  (4 MB)

    sums_pool = outer.enter_context(tc.tile_pool(name="sums", bufs=1))
    sums_acc = sums_pool.tile([P, 2 * IC], F32)  # per-row exp-sums (even cols)
    rsum = sums_pool.tile([P, 2 * IC], F32)

    # DRAM scratch for the full K^T / V (32 MB each) — streamed in phase 2.
    dram = outer.enter_context(tc.tile_pool(name="dram", bufs=1, space="DRAM"))
    kt_dram = dram.tile([DO, P, B], F32R)  # K^T: [o_out][o_in][j]
    v_dram = dram.tile([B, D], BF16)  # V: natural [j, o]

    # =========================================================
    # Phase 0+1: weight transposes, Q^T (local), K^T/V (full)
    # =========================================================
    with ExitStack() as p1:
        wt_pool = p1.enter_context(tc.tile_pool(name="wt", bufs=1))
        wqt = wt_pool.tile([P, DO, D], F32R)  # W^T: [d_in, d_out, o] (4 MB)
        wkt = wt_pool.tile([P, DO, D], F32R)
        wvt = wt_pool.tile([P, DO, D], BF16)

        row_pool = p1.enter_context(tc.tile_pool(name="rows", bufs=2))
        xt_pool = p1.enter_context(tc.tile_pool(name="xt", bufs=2))
        st_pool = p1.enter_context(tc.tile_pool(name="stage", bufs=2))
        tp_psum = p1.enter_context(tc.tile_pool(name="tp_ps", bufs=2, space="PSUM"))
        mm_psum = p1.enter_context(tc.tile_pool(name="mm_ps", bufs=4, space="PSUM"))

        # -- transpose the three weight matrices into SBUF --
        for wt_sb, w_dram in ((wqt, w_q), (wkt, w_k), (wvt, w_v)):
            for oo in range(DO):
                wrow = row_pool.tile([P, D], F32, tag="row")
                nc.sync.dma_start(wrow, w_dram[oo * P : (oo + 1) * P, :])
                for dd in range(DO):
                    tp = tp_psum.tile([P, P], F32, tag="tp")
                    nc.tensor.transpose(
                        tp, wrow[:, dd * P : (dd + 1) * P], identity
                    )
                    nc.vector.tensor_copy(
                        out=wt_sb[:, dd, oo * P : (oo + 1) * P], in_=tp
                    )

        # -- Q^T for the local shard, in halves of 512 rows --
        for ih in range(R // JBLK):
            xt_blk = xt_pool.tile([P, DO, JBLK], F32R, tag="xt")
            for jj in range(JBLK // P):
                xrow = row_pool.tile([P, D], F32, tag="row")
                nc.sync.dma_start(
                    xrow, x_loc[(ih * 4 + jj) * P : (ih * 4 + jj + 1) * P, :]
                )
                _transpose_rows_to_sbuf(nc, tp_psum, identity, xrow, xt_blk, jj)
            for oo in range(DO):
                pq = mm_psum.tile([P, JBLK], F32, tag="mm")
                for dd in range(DO):
                    nc.tensor.matmul(
                        pq,
                        (wqt[:, dd, oo * P : (oo + 1) * P]),
                        (xt_blk[:, dd, :]),
                        start=(dd == 0),
                        stop=(dd == DO - 1),
                    )
                nc.scalar.activation(
                    qt[:, oo, ih * JBLK : (ih + 1) * JBLK],
                    pq,
                    AF.Identity,
                    bias=bq_sb[:, oo : oo + 1],
                )

        # -- full K^T and V, streamed over 16 j-blocks of 512 rows --
        for jb in range(NJB):
            xt_blk = xt_pool.tile([P, DO, JBLK], F32R, tag="xt")
            for jj in range(JBLK // P):
                xrow = row_pool.tile([P, D], F32, tag="row")
                nc.sync.dma_start(
                    xrow, x[(jb * 4 + jj) * P : (jb * 4 + jj + 1) * P, :]
                )
                _transpose_rows_to_sbuf(nc, tp_psum, identity, xrow, xt_blk, jj)
            xt_bf = xt_pool.tile([P, DO, JBLK], BF16, tag="xtb")
            nc.vector.tensor_copy(out=xt_bf, in_=xt_blk)
            # K^T block: [o, j]
            for oo in range(DO):
                pk = mm_psum.tile([P, JBLK], F32, tag="mm")
                for dd in range(DO):
                    nc.tensor.matmul(
                        pk,
                        (wkt[:, dd, oo * P : (oo + 1) * P]),
                        (xt_blk[:, dd, :]),
                        start=(dd == 0),
                        stop=(dd == DO - 1),
                    )
                kst = st_pool.tile([P, JBLK], F32R, tag="kst")
                nc.scalar.activation(
                    kst, pk, AF.Identity, bias=bk_sb[:, oo : oo + 1]
                )
                nc.sync.dma_start(
                    kt_dram[oo, :, jb * JBLK : (jb + 1) * JBLK], kst
                )
            # V block: natural [j, o], bias deferred to the epilogue
            for jj in range(JBLK // P):
                vst = st_pool.tile([P, D], BF16, tag="vst")
                pv_h = [mm_psum.tile([P, JBLK], F32, tag="mm", name="pv") for _ in range(2)]
                for dd in range(DO):
                    for oh in range(2):
                        nc.tensor.matmul(
                            pv_h[oh],
                            (xt_bf[:, dd, jj * P : (jj + 1) * P]),
                            (wvt[:, dd, oh * 512 : (oh + 1) * 512]),
                            start=(dd == 0),
                            stop=(dd == DO - 1),
                        )
                for oh in range(2):
                    nc.vector.tensor_copy(
                        out=vst[:, oh * 512 : (oh + 1) * 512], in_=pv_h[oh]
                    )
                nc.sync.dma_start(
                    v_dram[jb * JBLK + jj * P : jb * JBLK + (jj + 1) * P, :], vst
                )

    # =========================================================
    # Phase 2: streamed attention in E^T layout
    # =========================================================
    with ExitStack() as p2:
        oa_pool = p2.enter_context(tc.tile_pool(name="oacc", bufs=1))
        outacc = oa_pool.tile([P, IC, D], F32)  # 4 MB

        kt_pool = p2.enter_context(tc.tile_pool(name="ktb", bufs=3))
        v_pool = p2.enter_context(tc.tile_pool(name="vtb", bufs=3))
        pt_pool = p2.enter_context(tc.tile_pool(name="ptb", bufs=3))
        e_psum = p2.enter_context(tc.tile_pool(name="e_ps", bufs=4, space="PSUM"))
        o_psum = p2.enter_context(tc.tile_pool(name="o_ps", bufs=3, space="PSUM"))
        s_psum = p2.enter_context(tc.tile_pool(name="s_ps", bufs=1, space="PSUM"))

        for jb in range(NJB):
            ktb = kt_pool.tile([P, DO, JBLK], F32R, tag="ktb")
            for oo in range(DO):
                nc.sync.dma_start(
                    ktb[:, oo, :], kt_dram[oo, :, jb * JBLK : (jb + 1) * JBLK]
                )
            vtb = v_pool.tile([P, JBLK // P, D], BF16, tag="vtb")
            nc.sync.dma_start(
                vtb,
                v_dram[jb * JBLK : (jb + 1) * JBLK, :].rearrange(
                    "(jj p) o -> p jj o", p=P
                ),
            )
            # unnormalized probabilities P^T for this j-block: [j, i]
            ptb = pt_pool.tile([P, JBLK // P, R], BF16, tag="ptb")
            for jj in range(JBLK // P):
                pe_h = [
                    e_psum.tile([P, JBLK], F32, tag="pe", name="pe")
                    for _ in range(R // JBLK)
                ]
                for oo in range(DO):
                    for ih in range(R // JBLK):
                        nc.tensor.matmul(
                            pe_h[ih],
                            (ktb[:, oo, jj * P : (jj + 1) * P]),
                            (qt[:, oo, ih * JBLK : (ih + 1) * JBLK]),
                            start=(oo == 0),
                            stop=(oo == DO - 1),
                        )
                for ih in range(R // JBLK):
                    nc.scalar.activation(
                        ptb[:, jj, ih * JBLK : (ih + 1) * JBLK],
                        pe_h[ih],
                        AF.Exp,
                        scale=SCALE,
                    )
            # row sums of P^T (reduce over j): matmul against ones
            # out_unnorm += P^T.T @ V, with the exp-sums matmul sharing each
            # stationary ptb tile (3 streams per weight load)
            ps = s_psum.tile([P, 2 * IC], F32, tag="ps")
            for ic in range(IC):
                po_h = [o_psum.tile([P, 512], F32, tag="po", name="po") for _ in range(2)]
                for jj in range(JBLK // P):
                    for oh in range(2):
                        nc.tensor.matmul(
                            po_h[oh],
                            (ptb[:, jj, ic * P : (ic + 1) * P]),
                            (vtb[:, jj, oh * 512 : (oh + 1) * 512]),
                            start=(jj == 0),
                            stop=(jj == JBLK // P - 1),
                        )
                    nc.tensor.matmul(
                        ps[:, 2 * ic : 2 * ic + 2],
                        (ptb[:, jj, ic * P : (ic + 1) * P]),
                        (ones),
                        start=(ic == 0 and jj == 0),
                        stop=(ic == IC - 1 and jj == JBLK // P - 1),
                    )
                for oh in range(2):
                    dst = outacc[:, ic, oh * 512 : (oh + 1) * 512]
                    if jb == 0:
                        nc.vector.tensor_copy(out=dst, in_=po_h[oh])
                    else:
                        nc.vector.tensor_tensor(dst, po_h[oh], dst, ALU.add)
            if jb == 0:
                nc.vector.tensor_copy(out=sums_acc, in_=ps)
            else:
                nc.vector.tensor_tensor(sums_acc, ps, sums_acc, ALU.add)

        # ---- epilogue: normalize, add bv, write out ----
        nc.vector.reciprocal(rsum, sums_acc)
        fin_pool = p2.enter_context(tc.tile_pool(name="fin", bufs=2))
        for ic in range(IC):
            ofin = fin_pool.tile([P, D], F32, tag="ofin")
            nc.vector.tensor_scalar_mul(ofin, outacc[:, ic, :], rsum[:, 2 * ic : 2 * ic + 1])
            nc.vector.tensor_tensor(ofin, ofin, bv_bc, ALU.add)
            nc.sync.dma_start(out_loc[ic * P : (ic + 1) * P, :], ofin)

    outer.close()


_NC_CACHE = None


def _get_program():
    global _NC_CACHE
    if _NC_CACHE is None:
        _NC_CACHE = build_program()
    return _NC_CACHE


def _run(inputs, trace=False):
    nc = _get_program()
    x = np.ascontiguousarray(np.asarray(inputs["x"], dtype=np.float32))
    common = {
        k: np.ascontiguousarray(np.asarray(inputs[k], dtype=np.float32))
        for k in ("Wq", "Wk", "Wv", "bq", "bk", "bv")
    }
    in_maps = [
        {"x": x, "x_loc": np.ascontiguousarray(x[c * R : (c + 1) * R]), **common}
        for c in range(NCORES)
    ]
    res = run_bass_kernel_spmd(
        nc, in_maps, core_ids=list(range(NCORES)), trace=trace
    )
    out = np.concatenate([res.results[c]["out_loc"] for c in range(NCORES)], axis=0)
    return out.reshape(B, D, 1).astype(np.float32), res


def kernel(**inputs):
    out, _ = _run(inputs, trace=False)
    return out



# revision 2
# speedup vs baseline: 1.4764x; 1.4764x over previous
"""Trainium2 Bass kernel for nn_MultiHeadAttention (B=8192, D=1024, 16 heads
used only via the softmax scale 1/8).

Strategy (8 NeuronCores, sharded projections + AllGather):
  - Rows (batch axis) are sharded: core c owns rows [c*1024, (c+1)*1024).
  - Each core computes Q^T, K^T and V for ONLY its local 1024 rows
    (6.4 GFLOP/core instead of the 36 GFLOP/core a replicated K/V
    projection would cost), then the K^T and V shards are AllGathered
    across the 8 cores. The collectives run on the TOPSP/SDMA silicon and
    overlap with the Q^T projection, so they are (nearly) free.
  - Attention runs in a transposed-energy ("E^T") layout so no probability
    transpose is ever needed:
        E^T[j, i] = sum_o K^T[o, j] * Q^T[o, i]
        P^T = exp(E^T * 0.125)            (no max subtraction; safe in f32)
        out_unnorm[i, o] = sum_j P^T[j, i] * V[j, o]
        s[i] = sum_j P^T[j, i]            (matmul against a ones vector)
        out = out_unnorm / s + bv         (bv folded in post-normalization)
  - All big matmuls run in float32r (full-rate streaming on the PE at
    N=512) with fp32 PSUM accumulation; P/V in bf16.
"""

import sys

sys.path.insert(0, "/opt/trn_rl_repo")

import numpy as np

import concourse.bass as bass  # noqa: F401
import concourse.tile as tile
from concourse import bacc, mybir
from concourse.bass_utils import run_bass_kernel_spmd
from concourse.masks import make_identity

B = 8192
D = 1024
P = 128
NCORES = 8
R = B // NCORES  # 1024 rows per core
JBLK = 512  # j-block (keys/values) streamed per iteration
NJB = B // JBLK  # 16
DO = D // P  # 8 feature chunks of 128
IC = R // P  # 8 row chunks of 128 per core
F32 = mybir.dt.float32
F32R = mybir.dt.float32r
BF16 = mybir.dt.bfloat16
AF = mybir.ActivationFunctionType
ALU = mybir.AluOpType
SCALE = 0.125  # 1/sqrt(head_dim=64)
RG = [list(range(NCORES))]


def _transpose_weight(nc, tp_psum, row_pool, identity, w_dram, wt_sb):
    """PE-transpose a [D, D] weight into the [128(d_in), DO, D(out)] SBUF
    layout (wt_sb[:, dd, o] = W[o, dd*128 + p])."""
    for oo in range(DO):
        wrow = row_pool.tile([P, D], F32, tag="row", name="wrow")
        nc.sync.dma_start(wrow, w_dram[oo * P : (oo + 1) * P, :])
        for dd in range(DO):
            tp = tp_psum.tile([P, P], F32, tag="tp", name="tp")
            nc.tensor.transpose(tp, wrow[:, dd * P : (dd + 1) * P], identity)
            nc.vector.tensor_copy(out=wt_sb[:, dd, oo * P : (oo + 1) * P], in_=tp)


def build_program():
    nc = bacc.Bacc(
        "TRN2", target_bir_lowering=False, debug=False, num_devices=NCORES
    )
    x_loc = nc.dram_tensor("x_loc", [R, D], F32, kind="ExternalInput").ap()
    w_q = nc.dram_tensor("Wq", [D, D], F32, kind="ExternalInput").ap()
    w_k = nc.dram_tensor("Wk", [D, D], F32, kind="ExternalInput").ap()
    w_v = nc.dram_tensor("Wv", [D, D], F32, kind="ExternalInput").ap()
    b_q = nc.dram_tensor("bq", [D], F32, kind="ExternalInput").ap()
    b_k = nc.dram_tensor("bk", [D], F32, kind="ExternalInput").ap()
    b_v = nc.dram_tensor("bv", [D], F32, kind="ExternalInput").ap()
    out_loc = nc.dram_tensor("out_loc", [R, D], F32, kind="ExternalOutput").ap()

    with tile.TileContext(nc) as tc:
        _body(nc, tc, x_loc, w_q, w_k, w_v, b_q, b_k, b_v, out_loc)
    nc.compile()
    return nc


def _body(nc, tc, x_loc, w_q, w_k, w_v, b_q, b_k, b_v, out_loc):
    from contextlib import ExitStack

    outer = ExitStack()
    outer.__enter__()
    # ---- persistent pools (whole kernel) ----
    const_pool = outer.enter_context(tc.tile_pool(name="const", bufs=1))
    identity = const_pool.tile([P, P], F32)
    make_identity(nc, identity)
    ones_f32 = const_pool.tile([P, 2], F32)
    nc.vector.memset(ones_f32, 1.0)
    ones = const_pool.tile([P, 2], BF16)
    nc.vector.tensor_copy(out=ones, in_=ones_f32)
    bq_sb = const_pool.tile([P, DO], F32)
    nc.sync.dma_start(bq_sb, b_q.rearrange("(oo p) -> p oo", p=P))
    bk_sb = const_pool.tile([P, DO], F32)
    nc.sync.dma_start(bk_sb, b_k.rearrange("(oo p) -> p oo", p=P))
    ones_row = const_pool.tile([1, P], F32)
    nc.vector.memset(ones_row, 1.0)
    # broadcast bv across all 128 partitions with a K=1 matmul:
    # load bv into partition 0 of bv_bc, then out[p, o] = 1 * bv[o]
    bv_bc = const_pool.tile([P, D], F32)
    nc.sync.dma_start(bv_bc[0:1, :], b_v[None, :])
    with tc.tile_pool(name="bv_psum", bufs=2, space="PSUM") as bvp:
        for oh in range(2):
            pt = bvp.tile([P, 512], F32, tag="bvp")
            nc.tensor.matmul(
                pt,
                ones_row,
                bv_bc[0:1, oh * 512 : (oh + 1) * 512],
                start=True,
                stop=True,
            )
            nc.vector.tensor_copy(out=bv_bc[:, oh * 512 : (oh + 1) * 512], in_=pt)

    qt_pool = outer.enter_context(tc.tile_pool(name="qt", bufs=1))
    qt = qt_pool.tile([P, DO, R], F32R)  # Q^T: [o_in, o_out, i]  (4 MB)

    sums_pool = outer.enter_context(tc.tile_pool(name="sums", bufs=1))
    sums_acc = sums_pool.tile([P, 2 * IC], F32)  # per-row exp-sums (even cols)
    rsum = sums_pool.tile([P, 2 * IC], F32)

    # DRAM scratch: local K^T/V shards (collective inputs) and the
    # AllGathered full K^T / V (collective outputs, streamed in phase 2).
    dram = outer.enter_context(tc.tile_pool(name="dram", bufs=1, space="DRAM"))
    kt_loc_d = dram.tile([DO, P, R], F32R)  # local K^T: [o_out][o_in][j]
    v_loc_d = dram.tile([R, D], BF16)  # local V: natural [j, o]
    kt_full_d = dram.tile([NCORES, DO, P, R], F32R, addr_space="Shared")
    v_full_d = dram.tile([NCORES, R, D], BF16, addr_space="Shared")

    # =========================================================
    # Phase 1: weight transposes, local x^T, local K^T/V/Q^T,
    #          AllGather of K^T and V (overlapped with Q^T)
    # =========================================================
    with ExitStack() as p1:
        wt_pool = p1.enter_context(tc.tile_pool(name="wt", bufs=1))
        wqt = wt_pool.tile([P, DO, D], F32R)  # W^T: [d_in, d_out, o] (4 MB)
        wkt = wt_pool.tile([P, DO, D], F32R)
        wvt = wt_pool.tile([P, DO, D], BF16)

        row_pool = p1.enter_context(tc.tile_pool(name="rows", bufs=2))
        xt_pool = p1.enter_context(tc.tile_pool(name="xt", bufs=1))
        st_pool = p1.enter_context(tc.tile_pool(name="stage", bufs=2))
        tp_psum = p1.enter_context(tc.tile_pool(name="tp_ps", bufs=2, space="PSUM"))
        mm_psum = p1.enter_context(tc.tile_pool(name="mm_ps", bufs=4, space="PSUM"))

        # -- transpose Wk; transpose the local x rows --
        _transpose_weight(nc, tp_psum, row_pool, identity, w_k, wkt)
        xt = xt_pool.tile([P, DO, R], F32R)  # x^T local: [d_in, d_out, i] 4MB
        xt_bf = xt_pool.tile([P, DO, R], BF16)
        for jj in range(IC):
            xrow = row_pool.tile([P, D], F32, tag="row", name="xrow")
            nc.sync.dma_start(xrow, x_loc[jj * P : (jj + 1) * P, :])
            for dd in range(DO):
                tp = tp_psum.tile([P, P], F32, tag="tp", name="tpx")
                nc.tensor.transpose(tp, xrow[:, dd * P : (dd + 1) * P], identity)
                nc.vector.tensor_copy(out=xt[:, dd, jj * P : (jj + 1) * P], in_=tp)
        nc.vector.tensor_copy(out=xt_bf, in_=xt)

        # -- local K^T ([o, j_local]), then AllGather it --
        for oo in range(DO):
            pk_h = [
                mm_psum.tile([P, JBLK], F32, tag="mm", name="pk") for _ in range(2)
            ]
            for dd in range(DO):
                for ih in range(2):
                    nc.tensor.matmul(
                        pk_h[ih],
                        (wkt[:, dd, oo * P : (oo + 1) * P]),
                        (xt[:, dd, ih * JBLK : (ih + 1) * JBLK]),
                        start=(dd == 0),
                        stop=(dd == DO - 1),
                    )
            for ih in range(2):
                kst = st_pool.tile([P, JBLK], F32R, tag="kst", name="kst")
                nc.scalar.activation(
                    kst, pk_h[ih], AF.Identity, bias=bk_sb[:, oo : oo + 1]
                )
                nc.sync.dma_start(
                    kt_loc_d[oo, :, ih * JBLK : (ih + 1) * JBLK], kst
                )
        nc.gpsimd.collective_compute(
            "AllGather",
            ALU.bypass,
            replica_groups=RG,
            ins=[kt_loc_d.opt()],
            outs=[kt_full_d.opt()],
        )

        # -- local V (natural [j_local, o], bias deferred), then AllGather --
        _transpose_weight(nc, tp_psum, row_pool, identity, w_v, wvt)
        for jj in range(IC):
            vst = st_pool.tile([P, D], BF16, tag="vst", name="vst")
            pv_h = [
                mm_psum.tile([P, JBLK], F32, tag="mm", name="pv") for _ in range(2)
            ]
            for dd in range(DO):
                for oh in range(2):
                    nc.tensor.matmul(
                        pv_h[oh],
                        (xt_bf[:, dd, jj * P : (jj + 1) * P]),
                        (wvt[:, dd, oh * 512 : (oh + 1) * 512]),
                        start=(dd == 0),
                        stop=(dd == DO - 1),
                    )
            for oh in range(2):
                nc.vector.tensor_copy(
                    out=vst[:, oh * 512 : (oh + 1) * 512], in_=pv_h[oh]
                )
            nc.sync.dma_start(v_loc_d[jj * P : (jj + 1) * P, :], vst)
        nc.gpsimd.collective_compute(
            "AllGather",
            ALU.bypass,
            replica_groups=RG,
            ins=[v_loc_d.opt()],
            outs=[v_full_d.opt()],
        )

        # -- local Q^T (overlaps the collectives) --
        _transpose_weight(nc, tp_psum, row_pool, identity, w_q, wqt)
        for oo in range(DO):
            pq_h = [
                mm_psum.tile([P, JBLK], F32, tag="mm", name="pq") for _ in range(2)
            ]
            for dd in range(DO):
                for ih in range(2):
                    nc.tensor.matmul(
                        pq_h[ih],
                        (wqt[:, dd, oo * P : (oo + 1) * P]),
                        (xt[:, dd, ih * JBLK : (ih + 1) * JBLK]),
                        start=(dd == 0),
                        stop=(dd == DO - 1),
                    )
            for ih in range(2):
                nc.scalar.activation(
                    qt[:, oo, ih * JBLK : (ih + 1) * JBLK],
                    pq_h[ih],
                    AF.Identity,
                    bias=bq_sb[:, oo : oo + 1],
                )

    # =========================================================
    # Phase 2: streamed attention in E^T layout
    # =========================================================
    with ExitStack() as p2:
        oa_pool = p2.enter_context(tc.tile_pool(name="oacc", bufs=1))
        outacc = oa_pool.tile([P, IC, D], F32)  # 4 MB

        kt_pool = p2.enter_context(tc.tile_pool(name="ktb", bufs=3))
        v_pool = p2.enter_context(tc.tile_pool(name="vtb", bufs=3))
        pt_pool = p2.enter_context(tc.tile_pool(name="ptb", bufs=3))
        e_psum = p2.enter_context(tc.tile_pool(name="e_ps", bufs=4, space="PSUM"))
        o_psum = p2.enter_context(tc.tile_pool(name="o_ps", bufs=3, space="PSUM"))
        s_psum = p2.enter_context(tc.tile_pool(name="s_ps", bufs=1, space="PSUM"))

        for jb in range(NJB):
            rr, off = jb // 2, (jb % 2) * JBLK
            ktb = kt_pool.tile([P, DO, JBLK], F32R, tag="ktb")
            for oo in range(DO):
                nc.sync.dma_start(
                    ktb[:, oo, :], kt_full_d[rr, oo, :, off : off + JBLK]
                )
            vtb = v_pool.tile([P, JBLK // P, D], BF16, tag="vtb")
            nc.sync.dma_start(
                vtb,
                v_full_d[rr, off : off + JBLK, :].rearrange(
                    "(jj p) o -> p jj o", p=P
                ),
            )
            # unnormalized probabilities P^T for this j-block: [j, i]
            ptb = pt_pool.tile([P, JBLK // P, R], BF16, tag="ptb")
            for jj in range(JBLK // P):
                pe_h = [
                    e_psum.tile([P, JBLK], F32, tag="pe", name="pe")
                    for _ in range(R // JBLK)
                ]
                for oo in range(DO):
                    for ih in range(R // JBLK):
                        nc.tensor.matmul(
                            pe_h[ih],
                            (ktb[:, oo, jj * P : (jj + 1) * P]),
                            (qt[:, oo, ih * JBLK : (ih + 1) * JBLK]),
                            start=(oo == 0),
                            stop=(oo == DO - 1),
                        )
                for ih in range(R // JBLK):
                    nc.scalar.activation(
                        ptb[:, jj, ih * JBLK : (ih + 1) * JBLK],
                        pe_h[ih],
                        AF.Exp,
                        scale=SCALE,
                    )
            # row sums of P^T (reduce over j): matmul against ones
            # out_unnorm += P^T.T @ V, with the exp-sums matmul sharing each
            # stationary ptb tile (3 streams per weight load)
            ps = s_psum.tile([P, 2 * IC], F32, tag="ps")
            for ic in range(IC):
                po_h = [o_psum.tile([P, 512], F32, tag="po", name="po") for _ in range(2)]
                for jj in range(JBLK // P):
                    for oh in range(2):
                        nc.tensor.matmul(
                            po_h[oh],
                            (ptb[:, jj, ic * P : (ic + 1) * P]),
                            (vtb[:, jj, oh * 512 : (oh + 1) * 512]),
                            start=(jj == 0),
                            stop=(jj == JBLK // P - 1),
                        )
                    nc.tensor.matmul(
                        ps[:, 2 * ic : 2 * ic + 2],
                        (ptb[:, jj, ic * P : (ic + 1) * P]),
                        (ones),
                        start=(ic == 0 and jj == 0),
                        stop=(ic == IC - 1 and jj == JBLK // P - 1),
                    )
                for oh in range(2):
                    dst = outacc[:, ic, oh * 512 : (oh + 1) * 512]
                    if jb == 0:
                        nc.vector.tensor_copy(out=dst, in_=po_h[oh])
                    else:
                        nc.vector.tensor_tensor(dst, po_h[oh], dst, ALU.add)
            if jb == 0:
                nc.vector.tensor_copy(out=sums_acc, in_=ps)
            else:
                nc.vector.tensor_tensor(sums_acc, ps, sums_acc, ALU.add)

        # ---- epilogue: normalize, add bv, write out ----
        nc.vector.reciprocal(rsum, sums_acc)
        fin_pool = p2.enter_context(tc.tile_pool(name="fin", bufs=2))
        for ic in range(IC):
            ofin = fin_pool.tile([P, D], F32, tag="ofin")
            nc.vector.tensor_scalar_mul(ofin, outacc[:, ic, :], rsum[:, 2 * ic : 2 * ic + 1])
            nc.vector.tensor_tensor(ofin, ofin, bv_bc, ALU.add)
            nc.sync.dma_start(out_loc[ic * P : (ic + 1) * P, :], ofin)

    outer.close()


_NC_CACHE = None


def _get_program():
    global _NC_CACHE
    if _NC_CACHE is None:
        _NC_CACHE = build_program()
    return _NC_CACHE


def _run(inputs, trace=False):
    nc = _get_program()
    x = np.ascontiguousarray(np.asarray(inputs["x"], dtype=np.float32))
    common = {
        k: np.ascontiguousarray(np.asarray(inputs[k], dtype=np.float32))
        for k in ("Wq", "Wk", "Wv", "bq", "bk", "bv")
    }
    in_maps = [
        {"x_loc": np.ascontiguousarray(x[c * R : (c + 1) * R]), **common}
        for c in range(NCORES)
    ]
    res = run_bass_kernel_spmd(
        nc, in_maps, core_ids=list(range(NCORES)), trace=trace
    )
    out = np.concatenate([res.results[c]["out_loc"] for c in range(NCORES)], axis=0)
    return out.reshape(B, D, 1).astype(np.float32), res


def kernel(**inputs):
    out, _ = _run(inputs, trace=False)
    return out


# revision 6
# speedup vs baseline: 1.5251x; 1.0329x over previous
"""Trainium2 Bass kernel for nn_MultiHeadAttention (B=8192, D=1024, 16 heads
used only via the softmax scale 1/8).

Strategy (8 NeuronCores, sharded projections + AllGather):
  - Rows (batch axis) are sharded: core c owns rows [c*1024, (c+1)*1024).
  - Each core computes Q^T, K^T and V for ONLY its local 1024 rows
    (6.4 GFLOP/core instead of the 36 GFLOP/core a replicated K/V
    projection would cost), then the K^T and V shards are AllGathered
    across the 8 cores. The collectives run on the TOPSP/SDMA silicon and
    overlap with the Q^T projection, so they are (nearly) free.
  - Attention runs in a transposed-energy ("E^T") layout so no probability
    transpose is ever needed:
        E^T[j, i] = sum_o K^T[o, j] * Q^T[o, i]
        P^T = exp(E^T * 0.125)            (no max subtraction; safe in f32)
        out_unnorm[i, o] = sum_j P^T[j, i] * V[j, o]
        s[i] = sum_j P^T[j, i]            (matmul against a ones vector)
        out = out_unnorm / s + bv         (bv folded in post-normalization)
  - All big matmuls run in float32r (full-rate streaming on the PE at
    N=512) with fp32 PSUM accumulation; P/V in bf16.
"""

import sys

sys.path.insert(0, "/opt/trn_rl_repo")

import numpy as np

import concourse.bass as bass  # noqa: F401
import concourse.tile as tile
from concourse import bacc, mybir
from concourse.bass_utils import run_bass_kernel_spmd
from concourse.masks import make_identity

B = 8192
D = 1024
P = 128
NCORES = 8
R = B // NCORES  # 1024 rows per core
JBLK = 512  # j-block (keys/values) streamed per iteration
NJB = B // JBLK  # 16
DO = D // P  # 8 feature chunks of 128
IC = R // P  # 8 row chunks of 128 per core
F32 = mybir.dt.float32
F32R = mybir.dt.float32r
BF16 = mybir.dt.bfloat16
AF = mybir.ActivationFunctionType
ALU = mybir.AluOpType
SCALE = 0.125  # 1/sqrt(head_dim=64)
RG = [list(range(NCORES))]


def _transpose_weight(nc, tp_psum, row_pool, identity, w_dram, wt_sb):
    """PE-transpose a [D, D] weight into the [128(d_in), DO, D(out)] SBUF
    layout (wt_sb[:, dd, o] = W[o, dd*128 + p])."""
    for oo in range(DO):
        wrow = row_pool.tile([P, D], F32, tag="row", name="wrow")
        nc.sync.dma_start(wrow, w_dram[oo * P : (oo + 1) * P, :])
        for dd in range(DO):
            tp = tp_psum.tile([P, P], F32, tag="tp", name="tp")
            nc.tensor.transpose(tp, wrow[:, dd * P : (dd + 1) * P], identity)
            nc.vector.tensor_copy(out=wt_sb[:, dd, oo * P : (oo + 1) * P], in_=tp)


def build_program():
    nc = bacc.Bacc(
        "TRN2", target_bir_lowering=False, debug=False, num_devices=NCORES
    )
    x_loc = nc.dram_tensor("x_loc", [R, D], F32, kind="ExternalInput").ap()
    w_q = nc.dram_tensor("Wq", [D, D], F32, kind="ExternalInput").ap()
    w_k = nc.dram_tensor("Wk", [D, D], F32, kind="ExternalInput").ap()
    w_v = nc.dram_tensor("Wv", [D, D], F32, kind="ExternalInput").ap()
    b_q = nc.dram_tensor("bq", [D], F32, kind="ExternalInput").ap()
    b_k = nc.dram_tensor("bk", [D], F32, kind="ExternalInput").ap()
    b_v = nc.dram_tensor("bv", [D], F32, kind="ExternalInput").ap()
    out_loc = nc.dram_tensor("out_loc", [R, D], F32, kind="ExternalOutput").ap()

    with tile.TileContext(nc) as tc:
        _body(nc, tc, x_loc, w_q, w_k, w_v, b_q, b_k, b_v, out_loc)
    nc.compile()
    return nc


def _body(nc, tc, x_loc, w_q, w_k, w_v, b_q, b_k, b_v, out_loc):
    from contextlib import ExitStack

    outer = ExitStack()
    outer.__enter__()
    # ---- persistent pools (whole kernel) ----
    const_pool = outer.enter_context(tc.tile_pool(name="const", bufs=1))
    identity = const_pool.tile([P, P], F32)
    make_identity(nc, identity)
    ones_f32 = const_pool.tile([P, 2], F32)
    nc.vector.memset(ones_f32, 1.0)
    ones = const_pool.tile([P, 2], BF16)
    nc.vector.tensor_copy(out=ones, in_=ones_f32)
    bq_sb = const_pool.tile([P, DO], F32)
    nc.sync.dma_start(bq_sb, b_q.rearrange("(oo p) -> p oo", p=P))
    bk_sb = const_pool.tile([P, DO], F32)
    nc.sync.dma_start(bk_sb, b_k.rearrange("(oo p) -> p oo", p=P))
    ones_row = const_pool.tile([1, P], F32)
    nc.vector.memset(ones_row, 1.0)
    # broadcast bv across all 128 partitions with a K=1 matmul:
    # load bv into partition 0 of bv_bc, then out[p, o] = 1 * bv[o]
    bv_bc = const_pool.tile([P, D], F32)
    nc.sync.dma_start(bv_bc[0:1, :], b_v[None, :])
    with tc.tile_pool(name="bv_psum", bufs=2, space="PSUM") as bvp:
        for oh in range(2):
            pt = bvp.tile([P, 512], F32, tag="bvp")
            nc.tensor.matmul(
                pt,
                ones_row,
                bv_bc[0:1, oh * 512 : (oh + 1) * 512],
                start=True,
                stop=True,
            )
            nc.vector.tensor_copy(out=bv_bc[:, oh * 512 : (oh + 1) * 512], in_=pt)

    qt_pool = outer.enter_context(tc.tile_pool(name="qt", bufs=1))
    qt = qt_pool.tile([P, DO, R], BF16)  # Q^T: [o_in, o_out, i]  (2 MB)

    sums_pool = outer.enter_context(tc.tile_pool(name="sums", bufs=1))
    sums_acc = sums_pool.tile([P, 2 * IC], F32)  # per-row exp-sums (even cols)
    rsum = sums_pool.tile([P, 2 * IC], F32)

    # DRAM scratch: local K^T+V shard (single collective input) and the
    # AllGathered full K^T+V (collective output, streamed in phase 2).
    # Slot 0 of axis 0 holds K^T [DO, P, R]; slot 1, viewed as
    # [(DO*P)=j, R=o], holds V in natural layout. Both bf16.
    dram = outer.enter_context(tc.tile_pool(name="dram", bufs=1, space="DRAM"))
    kv_loc_d = dram.tile([2, DO, P, R], BF16)
    kv_full_d = dram.tile([NCORES, 2, DO, P, R], BF16, addr_space="Shared")
    v_loc_view = kv_loc_d[1].rearrange("a p r -> (a p) r")

    # =========================================================
    # Phase 1: weight transposes, local x^T, local K^T/V/Q^T,
    #          AllGather of K^T and V (overlapped with Q^T)
    # =========================================================
    with ExitStack() as p1:
        wt_pool = p1.enter_context(tc.tile_pool(name="wt", bufs=1))
        wqt = wt_pool.tile([P, DO, D], F32R)  # W^T: [d_in, d_out, o] (4 MB)
        wkt = wt_pool.tile([P, DO, D], F32R)
        wvt = wt_pool.tile([P, DO, D], BF16)

        row_pool = p1.enter_context(tc.tile_pool(name="rows", bufs=2))
        xt_pool = p1.enter_context(tc.tile_pool(name="xt", bufs=1))
        st_pool = p1.enter_context(tc.tile_pool(name="stage", bufs=2))
        tp_psum = p1.enter_context(tc.tile_pool(name="tp_ps", bufs=2, space="PSUM"))
        mm_psum = p1.enter_context(tc.tile_pool(name="mm_ps", bufs=4, space="PSUM"))

        # -- transpose Wk; transpose the local x rows --
        _transpose_weight(nc, tp_psum, row_pool, identity, w_k, wkt)
        xt = xt_pool.tile([P, DO, R], F32R)  # x^T local: [d_in, d_out, i] 4MB
        xt_bf = xt_pool.tile([P, DO, R], BF16)
        for jj in range(IC):
            xrow = row_pool.tile([P, D], F32, tag="row", name="xrow")
            nc.sync.dma_start(xrow, x_loc[jj * P : (jj + 1) * P, :])
            for dd in range(DO):
                tp = tp_psum.tile([P, P], F32, tag="tp", name="tpx")
                nc.tensor.transpose(tp, xrow[:, dd * P : (dd + 1) * P], identity)
                nc.vector.tensor_copy(out=xt[:, dd, jj * P : (jj + 1) * P], in_=tp)
        nc.vector.tensor_copy(out=xt_bf, in_=xt)

        # -- local K^T ([o, j_local]) in bf16 --
        for oo in range(DO):
            pk_h = [
                mm_psum.tile([P, JBLK], F32, tag="mm", name="pk") for _ in range(2)
            ]
            for dd in range(DO):
                for ih in range(2):
                    nc.tensor.matmul(
                        pk_h[ih],
                        (wkt[:, dd, oo * P : (oo + 1) * P]),
                        (xt[:, dd, ih * JBLK : (ih + 1) * JBLK]),
                        start=(dd == 0),
                        stop=(dd == DO - 1),
                    )
            for ih in range(2):
                kst = st_pool.tile([P, JBLK], BF16, tag="kst", name="kst")
                nc.scalar.activation(
                    kst, pk_h[ih], AF.Identity, bias=bk_sb[:, oo : oo + 1]
                )
                nc.sync.dma_start(
                    kv_loc_d[0, oo, :, ih * JBLK : (ih + 1) * JBLK], kst
                )

        # -- local V (natural [j_local, o], bias deferred) --
        _transpose_weight(nc, tp_psum, row_pool, identity, w_v, wvt)
        for jj in range(IC):
            vst = st_pool.tile([P, D], BF16, tag="vst", name="vst")
            pv_h = [
                mm_psum.tile([P, JBLK], F32, tag="mm", name="pv") for _ in range(2)
            ]
            for dd in range(DO):
                for oh in range(2):
                    nc.tensor.matmul(
                        pv_h[oh],
                        (xt_bf[:, dd, jj * P : (jj + 1) * P]),
                        (wvt[:, dd, oh * 512 : (oh + 1) * 512]),
                        start=(dd == 0),
                        stop=(dd == DO - 1),
                    )
            for oh in range(2):
                nc.vector.tensor_copy(
                    out=vst[:, oh * 512 : (oh + 1) * 512], in_=pv_h[oh]
                )
            nc.sync.dma_start(v_loc_view[jj * P : (jj + 1) * P, :], vst)

        # -- one AllGather for the combined K^T+V shard (4 MB bf16) --
        nc.gpsimd.collective_compute(
            "AllGather",
            ALU.bypass,
            replica_groups=RG,
            ins=[kv_loc_d.opt()],
            outs=[kv_full_d.opt()],
        )

        # -- local Q^T (overlaps the collectives) --
        _transpose_weight(nc, tp_psum, row_pool, identity, w_q, wqt)
        for oo in range(DO):
            pq_h = [
                mm_psum.tile([P, JBLK], F32, tag="mm", name="pq") for _ in range(2)
            ]
            for dd in range(DO):
                for ih in range(2):
                    nc.tensor.matmul(
                        pq_h[ih],
                        (wqt[:, dd, oo * P : (oo + 1) * P]),
                        (xt[:, dd, ih * JBLK : (ih + 1) * JBLK]),
                        start=(dd == 0),
                        stop=(dd == DO - 1),
                    )
            for ih in range(2):
                nc.scalar.activation(
                    qt[:, oo, ih * JBLK : (ih + 1) * JBLK],
                    pq_h[ih],
                    AF.Identity,
                    bias=bq_sb[:, oo : oo + 1],
                )

    # =========================================================
    # Phase 2: streamed attention in E^T layout
    # =========================================================
    with ExitStack() as p2:
        oa_pool = p2.enter_context(tc.tile_pool(name="oacc", bufs=1))
        outacc = oa_pool.tile([P, IC, D], F32)  # 4 MB

        kt_pool = p2.enter_context(tc.tile_pool(name="ktb", bufs=3))
        v_pool = p2.enter_context(tc.tile_pool(name="vtb", bufs=3))
        pt_pool = p2.enter_context(tc.tile_pool(name="ptb", bufs=3))
        e_psum = p2.enter_context(tc.tile_pool(name="e_ps", bufs=4, space="PSUM"))
        o_psum = p2.enter_context(tc.tile_pool(name="o_ps", bufs=3, space="PSUM"))
        s_psum = p2.enter_context(tc.tile_pool(name="s_ps", bufs=1, space="PSUM"))

        for jb in range(NJB):
            rr, off = jb // 2, (jb % 2) * JBLK
            ktb = kt_pool.tile([P, DO, JBLK], BF16, tag="ktb")
            for oo in range(DO):
                nc.sync.dma_start(
                    ktb[:, oo, :], kv_full_d[rr, 0, oo, :, off : off + JBLK]
                )
            vtb = v_pool.tile([P, JBLK // P, D], BF16, tag="vtb")
            nc.sync.dma_start(
                vtb,
                kv_full_d[rr, 1]
                .rearrange("a p r -> (a p) r")[off : off + JBLK, :]
                .rearrange("(jj p) o -> p jj o", p=P),
            )
            # unnormalized probabilities P^T for this j-block: [j, i]
            ptb = pt_pool.tile([P, JBLK // P, R], BF16, tag="ptb")
            for jj in range(JBLK // P):
                pe_h = [
                    e_psum.tile([P, JBLK], F32, tag="pe", name="pe")
                    for _ in range(R // JBLK)
                ]
                for oo in range(DO):
                    for ih in range(R // JBLK):
                        nc.tensor.matmul(
                            pe_h[ih],
                            (ktb[:, oo, jj * P : (jj + 1) * P]),
                            (qt[:, oo, ih * JBLK : (ih + 1) * JBLK]),
                            start=(oo == 0),
                            stop=(oo == DO - 1),
                        )
                for ih in range(R // JBLK):
                    nc.scalar.activation(
                        ptb[:, jj, ih * JBLK : (ih + 1) * JBLK],
                        pe_h[ih],
                        AF.Exp,
                        scale=SCALE,
                    )
            # row sums of P^T (reduce over j): matmul against ones
            # out_unnorm += P^T.T @ V, with the exp-sums matmul sharing each
            # stationary ptb tile (3 streams per weight load)
            ps = s_psum.tile([P, 2 * IC], F32, tag="ps")
            for ic in range(IC):
                po_h = [o_psum.tile([P, 512], F32, tag="po", name="po") for _ in range(2)]
                for jj in range(JBLK // P):
                    for oh in range(2):
                        nc.tensor.matmul(
                            po_h[oh],
                            (ptb[:, jj, ic * P : (ic + 1) * P]),
                            (vtb[:, jj, oh * 512 : (oh + 1) * 512]),
                            start=(jj == 0),
                            stop=(jj == JBLK // P - 1),
                        )
                    nc.tensor.matmul(
                        ps[:, 2 * ic : 2 * ic + 2],
                        (ptb[:, jj, ic * P : (ic + 1) * P]),
                        (ones),
                        start=(ic == 0 and jj == 0),
                        stop=(ic == IC - 1 and jj == JBLK // P - 1),
                    )
                for oh in range(2):
                    dst = outacc[:, ic, oh * 512 : (oh + 1) * 512]
                    if jb == 0:
                        nc.vector.tensor_copy(out=dst, in_=po_h[oh])
                    else:
                        nc.vector.tensor_tensor(dst, po_h[oh], dst, ALU.add)
            if jb == 0:
                nc.vector.tensor_copy(out=sums_acc, in_=ps)
            else:
                nc.vector.tensor_tensor(sums_acc, ps, sums_acc, ALU.add)

        # ---- epilogue: normalize, add bv, write out ----
        nc.vector.reciprocal(rsum, sums_acc)
        fin_pool = p2.enter_context(tc.tile_pool(name="fin", bufs=2))
        for ic in range(IC):
            ofin = fin_pool.tile([P, D], F32, tag="ofin")
            nc.vector.tensor_scalar_mul(ofin, outacc[:, ic, :], rsum[:, 2 * ic : 2 * ic + 1])
            nc.vector.tensor_tensor(ofin, ofin, bv_bc, ALU.add)
            nc.sync.dma_start(out_loc[ic * P : (ic + 1) * P, :], ofin)

    outer.close()


_NC_CACHE = None


def _get_program():
    global _NC_CACHE
    if _NC_CACHE is None:
        _NC_CACHE = build_program()
    return _NC_CACHE


def _run(inputs, trace=False):
    nc = _get_program()
    x = np.ascontiguousarray(np.asarray(inputs["x"], dtype=np.float32))
    common = {
        k: np.ascontiguousarray(np.asarray(inputs[k], dtype=np.float32))
        for k in ("Wq", "Wk", "Wv", "bq", "bk", "bv")
    }
    in_maps = [
        {"x_loc": np.ascontiguousarray(x[c * R : (c + 1) * R]), **common}
        for c in range(NCORES)
    ]
    res = run_bass_kernel_spmd(
        nc, in_maps, core_ids=list(range(NCORES)), trace=trace
    )
    out = np.concatenate([res.results[c]["out_loc"] for c in range(NCORES)], axis=0)
    return out.reshape(B, D, 1).astype(np.float32), res


def kernel(**inputs):
    out, _ = _run(inputs, trace=False)
    return out


# revision 7
# speedup vs baseline: 1.6222x; 1.0637x over previous
"""Trainium2 Bass kernel for nn_MultiHeadAttention (B=8192, D=1024, 16 heads
used only via the softmax scale 1/8).

Strategy (8 NeuronCores, sharded projections + pipelined AllGather):
  - Rows (batch axis) are sharded: core c owns rows [c*1024, (c+1)*1024).
  - Each core computes Q^T, K^T and V for ONLY its local 1024 rows
    (6.4 GFLOP/core instead of the 36 GFLOP/core a replicated K/V
    projection would cost). The K^T and V shards are AllGathered in TWO
    halves (key rows 0:512 and 512:1024 of each rank, bf16), triggered
    as soon as each half is computed so the collective (TOPSP/SDMA
    silicon) overlaps the rest of the projection work and Q^T.
  - Attention runs in a transposed-energy ("E^T") layout so no probability
    transpose is ever needed:
        E^T[j, i] = sum_o K^T[o, j] * Q^T[o, i]
        P^T = exp(E^T * 0.125)            (no max subtraction; safe in f32)
        out_unnorm[i, o] = sum_j P^T[j, i] * V[j, o]
        s[i] = sum_j P^T[j, i]            (matmul against a ones vector)
        out = out_unnorm / s + bv         (bv folded in post-normalization)
  - K^T travels bf16 over the wire but is upcast to f32r on load so the
    E^T matmul runs f32r x f32r; P/V stay bf16.
  - Phase 2 processes the off=0 half of every rank first (unlocked by the
    first AllGather), then the off=512 half.
"""

import sys

sys.path.insert(0, "/opt/trn_rl_repo")

import numpy as np

import concourse.bass as bass  # noqa: F401
import concourse.tile as tile
from concourse import bacc, mybir
from concourse.bass_utils import run_bass_kernel_spmd
from concourse.masks import make_identity

B = 8192
D = 1024
P = 128
NCORES = 8
R = B // NCORES  # 1024 rows per core
JBLK = 512  # j-block (keys/values) streamed per iteration
NJB = B // JBLK  # 16
DO = D // P  # 8 feature chunks of 128
IC = R // P  # 8 row chunks of 128 per core
F32 = mybir.dt.float32
F32R = mybir.dt.float32r
BF16 = mybir.dt.bfloat16
AF = mybir.ActivationFunctionType
ALU = mybir.AluOpType
SCALE = 0.125  # 1/sqrt(head_dim=64)
RG = [list(range(NCORES))]
HALF = DO * P * JBLK  # flat bf16 elements of one K or V half (512K)


def _transpose_weight(nc, tp_psum, row_pool, identity, w_dram, wt_sb):
    """PE-transpose a [D, D] weight into the [128(d_in), DO, D(out)] SBUF
    layout (wt_sb[:, dd, o] = W[o, dd*128 + p])."""
    for oo in range(DO):
        wrow = row_pool.tile([P, D], F32, tag="row", name="wrow")
        nc.sync.dma_start(wrow, w_dram[oo * P : (oo + 1) * P, :])
        for dd in range(DO):
            tp = tp_psum.tile([P, P], F32, tag="tp", name="tp")
            nc.tensor.transpose(tp, wrow[:, dd * P : (dd + 1) * P], identity)
            nc.vector.tensor_copy(out=wt_sb[:, dd, oo * P : (oo + 1) * P], in_=tp)


def build_program():
    nc = bacc.Bacc(
        "TRN2", target_bir_lowering=False, debug=False, num_devices=NCORES
    )
    x_loc = nc.dram_tensor("x_loc", [R, D], F32, kind="ExternalInput").ap()
    w_q = nc.dram_tensor("Wq", [D, D], F32, kind="ExternalInput").ap()
    w_k = nc.dram_tensor("Wk", [D, D], F32, kind="ExternalInput").ap()
    w_v = nc.dram_tensor("Wv", [D, D], F32, kind="ExternalInput").ap()
    b_q = nc.dram_tensor("bq", [D], F32, kind="ExternalInput").ap()
    b_k = nc.dram_tensor("bk", [D], F32, kind="ExternalInput").ap()
    b_v = nc.dram_tensor("bv", [D], F32, kind="ExternalInput").ap()
    out_loc = nc.dram_tensor("out_loc", [R, D], F32, kind="ExternalOutput").ap()

    with tile.TileContext(nc) as tc:
        _body(nc, tc, x_loc, w_q, w_k, w_v, b_q, b_k, b_v, out_loc)
    nc.compile()
    return nc


def _body(nc, tc, x_loc, w_q, w_k, w_v, b_q, b_k, b_v, out_loc):
    from contextlib import ExitStack

    outer = ExitStack()
    outer.__enter__()
    # ---- persistent pools (whole kernel) ----
    const_pool = outer.enter_context(tc.tile_pool(name="const", bufs=1))
    identity = const_pool.tile([P, P], F32)
    make_identity(nc, identity)
    ones_f32 = const_pool.tile([P, 2], F32)
    nc.vector.memset(ones_f32, 1.0)
    ones = const_pool.tile([P, 2], BF16)
    nc.vector.tensor_copy(out=ones, in_=ones_f32)
    bq_sb = const_pool.tile([P, DO], F32)
    nc.sync.dma_start(bq_sb, b_q.rearrange("(oo p) -> p oo", p=P))
    bk_sb = const_pool.tile([P, DO], F32)
    nc.sync.dma_start(bk_sb, b_k.rearrange("(oo p) -> p oo", p=P))
    ones_row = const_pool.tile([1, P], F32)
    nc.vector.memset(ones_row, 1.0)
    # broadcast bv across all 128 partitions with a K=1 matmul:
    # load bv into partition 0 of bv_bc, then out[p, o] = 1 * bv[o]
    bv_bc = const_pool.tile([P, D], F32)
    nc.sync.dma_start(bv_bc[0:1, :], b_v[None, :])
    with tc.tile_pool(name="bv_psum", bufs=2, space="PSUM") as bvp:
        for oh in range(2):
            pt = bvp.tile([P, 512], F32, tag="bvp")
            nc.tensor.matmul(
                pt,
                ones_row,
                bv_bc[0:1, oh * 512 : (oh + 1) * 512],
                start=True,
                stop=True,
            )
            nc.vector.tensor_copy(out=bv_bc[:, oh * 512 : (oh + 1) * 512], in_=pt)

    qt_pool = outer.enter_context(tc.tile_pool(name="qt", bufs=1))
    qt = qt_pool.tile([P, DO, R], F32R)  # Q^T: [o_in, o_out, i]  (4 MB)

    sums_pool = outer.enter_context(tc.tile_pool(name="sums", bufs=1))
    sums_acc = sums_pool.tile([P, 2 * IC], F32)  # per-row exp-sums (even cols)
    rsum = sums_pool.tile([P, 2 * IC], F32)

    # DRAM scratch: per-half local K^T+V shard (collective inputs) and the
    # AllGathered halves. Each half is [2, HALF] bf16: slot 0 = K^T
    # [DO, P, JBLK] flattened, slot 1 = V rows [JBLK, D] flattened.
    dram = outer.enter_context(tc.tile_pool(name="dram", bufs=1, space="DRAM"))
    kv_loc_h = [dram.tile([2, HALF], BF16, name=f"kv_loc_{h}") for h in range(2)]
    kv_full_h = [
        dram.tile([NCORES, 2, HALF], BF16, addr_space="Shared", name=f"kv_full_{h}")
        for h in range(2)
    ]

    def k_dst(h, oo):  # [P, JBLK] write view of K^T half h, feature chunk oo
        return kv_loc_h[h][0].rearrange("(a p r) -> a p r", a=DO, p=P)[oo]

    def v_dst(h, jv):  # [P, D] write view of V half h, row chunk jv
        return kv_loc_h[h][1].rearrange("(j o) -> j o", o=D)[
            jv * P : (jv + 1) * P, :
        ]

    # =========================================================
    # Phase 1: weight transposes, local x^T, local K^T/V/Q^T,
    #          two pipelined AllGathers of the K^T+V halves
    # =========================================================
    with ExitStack() as p1:
        wt_pool = p1.enter_context(tc.tile_pool(name="wt", bufs=1))
        wqt = wt_pool.tile([P, DO, D], F32R)  # W^T: [d_in, d_out, o] (4 MB)
        wkt = wt_pool.tile([P, DO, D], F32R)
        wvt = wt_pool.tile([P, DO, D], BF16)

        row_pool = p1.enter_context(tc.tile_pool(name="rows", bufs=2))
        xt_pool = p1.enter_context(tc.tile_pool(name="xt", bufs=1))
        st_pool = p1.enter_context(tc.tile_pool(name="stage", bufs=2))
        tp_psum = p1.enter_context(tc.tile_pool(name="tp_ps", bufs=2, space="PSUM"))
        mm_psum = p1.enter_context(tc.tile_pool(name="mm_ps", bufs=4, space="PSUM"))

        # -- transpose Wk; transpose the local x rows --
        _transpose_weight(nc, tp_psum, row_pool, identity, w_k, wkt)
        xt = xt_pool.tile([P, DO, R], F32R)  # x^T local: [d_in, d_out, i] 4MB
        xt_bf = xt_pool.tile([P, DO, R], BF16)
        for jj in range(IC):
            xrow = row_pool.tile([P, D], F32, tag="row", name="xrow")
            nc.sync.dma_start(xrow, x_loc[jj * P : (jj + 1) * P, :])
            for dd in range(DO):
                tp = tp_psum.tile([P, P], F32, tag="tp", name="tpx")
                nc.tensor.transpose(tp, xrow[:, dd * P : (dd + 1) * P], identity)
                nc.vector.tensor_copy(out=xt[:, dd, jj * P : (jj + 1) * P], in_=tp)
        nc.vector.tensor_copy(out=xt_bf, in_=xt)
        _transpose_weight(nc, tp_psum, row_pool, identity, w_v, wvt)

        # -- per half: local K^T columns, local V rows, then AllGather --
        for h in range(2):
            for oo in range(DO):
                pk = mm_psum.tile([P, JBLK], F32, tag="mm", name="pk")
                for dd in range(DO):
                    nc.tensor.matmul(
                        pk,
                        (wkt[:, dd, oo * P : (oo + 1) * P]),
                        (xt[:, dd, h * JBLK : (h + 1) * JBLK]),
                        start=(dd == 0),
                        stop=(dd == DO - 1),
                    )
                kst = st_pool.tile([P, JBLK], BF16, tag="kst", name="kst")
                nc.scalar.activation(
                    kst, pk, AF.Identity, bias=bk_sb[:, oo : oo + 1]
                )
                nc.sync.dma_start(k_dst(h, oo), kst)
            for jv in range(JBLK // P):
                jj = h * (JBLK // P) + jv
                vst = st_pool.tile([P, D], BF16, tag="vst", name="vst")
                pv_h = [
                    mm_psum.tile([P, JBLK], F32, tag="mm", name="pv")
                    for _ in range(2)
                ]
                for dd in range(DO):
                    for oh in range(2):
                        nc.tensor.matmul(
                            pv_h[oh],
                            (xt_bf[:, dd, jj * P : (jj + 1) * P]),
                            (wvt[:, dd, oh * 512 : (oh + 1) * 512]),
                            start=(dd == 0),
                            stop=(dd == DO - 1),
                        )
                for oh in range(2):
                    nc.vector.tensor_copy(
                        out=vst[:, oh * 512 : (oh + 1) * 512], in_=pv_h[oh]
                    )
                nc.sync.dma_start(v_dst(h, jv), vst)
            nc.gpsimd.collective_compute(
                "AllGather",
                ALU.bypass,
                replica_groups=RG,
                ins=[kv_loc_h[h].opt()],
                outs=[kv_full_h[h].opt()],
            )

        # -- local Q^T (overlaps the collectives) --
        _transpose_weight(nc, tp_psum, row_pool, identity, w_q, wqt)
        for oo in range(DO):
            pq_h = [
                mm_psum.tile([P, JBLK], F32, tag="mm", name="pq") for _ in range(2)
            ]
            for dd in range(DO):
                for ih in range(2):
                    nc.tensor.matmul(
                        pq_h[ih],
                        (wqt[:, dd, oo * P : (oo + 1) * P]),
                        (xt[:, dd, ih * JBLK : (ih + 1) * JBLK]),
                        start=(dd == 0),
                        stop=(dd == DO - 1),
                    )
            for ih in range(2):
                nc.scalar.activation(
                    qt[:, oo, ih * JBLK : (ih + 1) * JBLK],
                    pq_h[ih],
                    AF.Identity,
                    bias=bq_sb[:, oo : oo + 1],
                )

    # =========================================================
    # Phase 2: streamed attention in E^T layout; first the off=0
    # half of every rank (AG half 0), then the off=512 half.
    # =========================================================
    with ExitStack() as p2:
        oa_pool = p2.enter_context(tc.tile_pool(name="oacc", bufs=1))
        outacc = oa_pool.tile([P, IC, D], F32)  # 4 MB

        ktb_bf_pool = p2.enter_context(tc.tile_pool(name="ktbb", bufs=2))
        kt_pool = p2.enter_context(tc.tile_pool(name="ktb", bufs=2))
        v_pool = p2.enter_context(tc.tile_pool(name="vtb", bufs=3))
        pt_pool = p2.enter_context(tc.tile_pool(name="ptb", bufs=3))
        e_psum = p2.enter_context(tc.tile_pool(name="e_ps", bufs=4, space="PSUM"))
        o_psum = p2.enter_context(tc.tile_pool(name="o_ps", bufs=3, space="PSUM"))
        s_psum = p2.enter_context(tc.tile_pool(name="s_ps", bufs=1, space="PSUM"))

        for jb in range(NJB):
            h, rr = jb // NCORES, jb % NCORES
            base = kv_full_h[h][rr]  # [2, HALF]
            ktb_bf = ktb_bf_pool.tile([P, DO, JBLK], BF16, tag="ktbb")
            nc.sync.dma_start(
                ktb_bf,
                base[0].rearrange("(a p r) -> p a r", a=DO, p=P),
            )
            ktb = kt_pool.tile([P, DO, JBLK], F32R, tag="ktb")
            nc.vector.tensor_copy(out=ktb, in_=ktb_bf)
            vtb = v_pool.tile([P, JBLK // P, D], BF16, tag="vtb")
            nc.sync.dma_start(
                vtb,
                base[1].rearrange("(jj p o) -> p jj o", p=P, o=D),
            )
            # unnormalized probabilities P^T for this j-block: [j, i]
            ptb = pt_pool.tile([P, JBLK // P, R], BF16, tag="ptb")
            for jj in range(JBLK // P):
                pe_h = [
                    e_psum.tile([P, JBLK], F32, tag="pe", name="pe")
                    for _ in range(R // JBLK)
                ]
                for oo in range(DO):
                    for ih in range(R // JBLK):
                        nc.tensor.matmul(
                            pe_h[ih],
                            (ktb[:, oo, jj * P : (jj + 1) * P]),
                            (qt[:, oo, ih * JBLK : (ih + 1) * JBLK]),
                            start=(oo == 0),
                            stop=(oo == DO - 1),
                        )
                for ih in range(R // JBLK):
                    nc.scalar.activation(
                        ptb[:, jj, ih * JBLK : (ih + 1) * JBLK],
                        pe_h[ih],
                        AF.Exp,
                        scale=SCALE,
                    )
            # row sums of P^T (reduce over j): matmul against ones
            # out_unnorm += P^T.T @ V, with the exp-sums matmul sharing each
            # stationary ptb tile (3 streams per weight load)
            ps = s_psum.tile([P, 2 * IC], F32, tag="ps")
            for ic in range(IC):
                po_h = [o_psum.tile([P, 512], F32, tag="po", name="po") for _ in range(2)]
                for jj in range(JBLK // P):
                    for oh in range(2):
                        nc.tensor.matmul(
                            po_h[oh],
                            (ptb[:, jj, ic * P : (ic + 1) * P]),
                            (vtb[:, jj, oh * 512 : (oh + 1) * 512]),
                            start=(jj == 0),
                            stop=(jj == JBLK // P - 1),
                        )
                    nc.tensor.matmul(
                        ps[:, 2 * ic : 2 * ic + 2],
                        (ptb[:, jj, ic * P : (ic + 1) * P]),
                        (ones),
                        start=(ic == 0 and jj == 0),
                        stop=(ic == IC - 1 and jj == JBLK // P - 1),
                    )
                for oh in range(2):
                    dst = outacc[:, ic, oh * 512 : (oh + 1) * 512]
                    if jb == 0:
                        nc.vector.tensor_copy(out=dst, in_=po_h[oh])
                    else:
                        nc.vector.tensor_tensor(dst, po_h[oh], dst, ALU.add)
            if jb == 0:
                nc.vector.tensor_copy(out=sums_acc, in_=ps)
            else:
                nc.vector.tensor_tensor(sums_acc, ps, sums_acc, ALU.add)

        # ---- epilogue: normalize, add bv, write out ----
        nc.vector.reciprocal(rsum, sums_acc)
        fin_pool = p2.enter_context(tc.tile_pool(name="fin", bufs=2))
        for ic in range(IC):
            ofin = fin_pool.tile([P, D], F32, tag="ofin")
            nc.vector.tensor_scalar_mul(ofin, outacc[:, ic, :], rsum[:, 2 * ic : 2 * ic + 1])
            nc.vector.tensor_tensor(ofin, ofin, bv_bc, ALU.add)
            nc.sync.dma_start(out_loc[ic * P : (ic + 1) * P, :], ofin)

    outer.close()


_NC_CACHE = None


def _get_program():
    global _NC_CACHE
    if _NC_CACHE is None:
        _NC_CACHE = build_program()
    return _NC_CACHE


def _run(inputs, trace=False):
    nc = _get_program()
    x = np.ascontiguousarray(np.asarray(inputs["x"], dtype=np.float32))
    common = {
        k: np.ascontiguousarray(np.asarray(inputs[k], dtype=np.float32))
        for k in ("Wq", "Wk", "Wv", "bq", "bk", "bv")
    }
    in_maps = [
        {"x_loc": np.ascontiguousarray(x[c * R : (c + 1) * R]), **common}
        for c in range(NCORES)
    ]
    res = run_bass_kernel_spmd(
        nc, in_maps, core_ids=list(range(NCORES)), trace=trace
    )
    out = np.concatenate([res.results[c]["out_loc"] for c in range(NCORES)], axis=0)
    return out.reshape(B, D, 1).astype(np.float32), res


def kernel(**inputs):
    out, _ = _run(inputs, trace=False)
    return out
